# revision 18
# baseline (speedup 1.0000x reference)
"""AttnBlock (GroupNorm -> single-head attention over 64x64 tokens -> proj -> residual)
for Trainium2, SPMD over 8 NeuronCores.

Sharding: core = batch(4) x query-half(2).  Each core receives x[b] with its
query half rotated to the front (token order along j is permutation-invariant
for softmax-attention and for GroupNorm stats), computes GroupNorm + k/vT over
all 4096 tokens, q over its 2048 tokens, streaming-softmax attention without
max-subtraction, and the output projection + residual for its 2048 tokens.

All matmuls run in fp8(e4m3) with DoubleRow perf mode: each instruction
contracts 2x128=256 inputs at 0.5 cycles/row -> 4x bf16 matmul throughput.
Scales (all exact powers of two, folded away):
  weights stored as 16*W^T fp8; q,k stored as 16*q (bias 16*b folded in)
  S_psum = 256*(q.k);  et = exp(S/sqrt(C) - ln16)   (max ~92 < 240 fp8e4 max)
  l_psum = 0.25*sum(et) = sum(e^S)/64; lrb = recip = 64/sum(e^S) broadcast
           to 128 partitions via a tiny ones-matmul (no DRAM roundtrip)
  o8 = O_psum*lrb = 64*(attention out, pre-proj); v has NO bias on device --
       bv is folded host-side into bp' = bp + Wp@bv
  proj_psum = (16Wp)*(o8) = 1024*h_attn;  y = proj*2^-10 + (x + bp')
Residual uses the bf16 x already in SBUF (no f32 x load).

Engine budget per core (cost-model): PE ~82us (was 303), ACT = exp only
~77us, DVE ~60us (GN stats + k-bias + O-normalize), Pool ~46us (h8/v/q
drains + residual combine), SP = DMA ~22us.
"""

import math
import numpy as np
import ml_dtypes

import concourse.bass as bass
import concourse.mybir as mybir
import concourse.tile as tile

P = 128
C = 512
NCC = C // P          # 4 channel chunks
NP2 = NCC // 2        # 2 channel-chunk pairs (DoubleRow)
HW = 4096             # tokens per batch image
IHALF = 2048          # query tokens per core
NBLK = IHALF // 512   # 4 i-blocks of 512
NJC = HW // P         # 32 j chunks of 128
NJP = NJC // 2        # 16 j-chunk pairs
NJT = HW // 512       # 8 j tiles of 512
GS = 16               # channels per group
EPS = 1e-6
WS = 16.0             # host-side weight scale (power of two)
SCALE_S = 1.0 / (WS * WS * math.sqrt(C))
EXP_BIAS = -math.log(16.0)
ONES_VAL = 0.25       # l_psum = sum(e^S)/64 -> recip = 64/sum = o8 scale
PROJ_SCALE = 1.0 / 1024.0

F32 = mybir.dt.float32
BF16 = mybir.dt.bfloat16
FP8 = mybir.dt.float8e4
BF = ml_dtypes.bfloat16
E4 = ml_dtypes.float8_e4m3
DR = mybir.MatmulPerfMode.DoubleRow
ALU = mybir.AluOpType
ACTF = mybir.ActivationFunctionType


def _split_excess_waits(nc):
    """walrus in this container accepts only ONE sync-wait per instruction;
    move extra waits onto same-engine NOPs placed immediately before."""
    for fn in nc.m.functions:
        for bb in fn.blocks:
            insts = list(bb.instructions)
            out = []
            changed = False
            for inst in insts:
                si = inst.sync_info
                if si is not None and len(si.on_wait) > 1:
                    waits = list(si.on_wait)
                    for k, w in enumerate(waits[:-1]):
                        nop = mybir.InstNoOp(
                            name=f"{inst.name}-ws{k}",
                            sync_info=mybir.SyncInfo(on_wait=[w], on_update=[]),
                            bass_nofuse=True,
                            engine=inst.engine,
                        )
                        out.append(nop)
                    inst.sync_info = mybir.SyncInfo(
                        on_wait=[waits[-1]], on_update=list(si.on_update)
                    )
                    changed = True
                out.append(inst)
            if changed:
                bb.instructions = out


def build_nc(split_waits=True):
    nc = bass.Bass()

    x_d = nc.declare_dram_parameter("x_bf", [C, HW], BF16, isOutput=False)
    wq_d = nc.declare_dram_parameter("wq8", [C, C], FP8, isOutput=False)
    wk_d = nc.declare_dram_parameter("wk8", [C, C], FP8, isOutput=False)
    wv_d = nc.declare_dram_parameter("wv8", [C, C], FP8, isOutput=False)
    wp_d = nc.declare_dram_parameter("wp8", [C, C], FP8, isOutput=False)
    # packed per-channel constants: bq16, bk16, bp', gamma, beta (NCC each)
    # then ind16 (P//GS cols)
    consts_d = nc.declare_dram_parameter("consts", [P, 5 * NCC + P // GS], F32,
                                         isOutput=False)
    bcast16_d = nc.declare_dram_parameter("bcast16", [P // GS, P], F32,
                                          isOutput=False)
    ones8_d = nc.declare_dram_parameter("ones8", [P, 2, 1], FP8, isOutput=False)
    y_d = nc.declare_dram_parameter("yout", [C, IHALF], F32, isOutput=True)

    with tile.TileContext(nc) as tc:
        with (
            tc.tile_pool(name="w", bufs=1) as wpool,
            tc.tile_pool(name="const", bufs=1) as cpool,
            tc.tile_pool(name="xb", bufs=1) as xpool,
            tc.tile_pool(name="h8p", bufs=1) as hpool,
            tc.tile_pool(name="k8p", bufs=1) as kpool,
            tc.tile_pool(name="q8p", bufs=1) as qpool,
            tc.tile_pool(name="v8p", bufs=1) as vpool,
        ):
            wq8 = wpool.tile([P, NCC, C], FP8, tag="wq8")
            wk8 = wpool.tile([P, NCC, C], FP8, tag="wk8")
            wv8 = wpool.tile([P, NCC, C], FP8, tag="wv8")
            wp8 = wpool.tile([P, NCC, C], FP8, tag="wp8")

            consts = cpool.tile([P, 5 * NCC + P // GS], F32, tag="consts")
            bq16 = consts[:, 0 * NCC:1 * NCC]
            bk16 = consts[:, 1 * NCC:2 * NCC]
            bppc = consts[:, 2 * NCC:3 * NCC]
            gamma = consts[:, 3 * NCC:4 * NCC]
            beta = consts[:, 4 * NCC:5 * NCC]
            ind16 = consts[:, 5 * NCC:]
            bcast16 = cpool.tile([P // GS, P], F32, tag="bcast16")
            ones8 = cpool.tile([P, 2, 1], FP8, tag="ones8")
            ones_bf = cpool.tile([1, P], BF16, tag="onesbf")
            eps_sb = cpool.tile([P // GS, 1], F32, tag="eps")
            ebias = cpool.tile([P, 1], F32, tag="ebias")

            x_sb = xpool.tile([P, NCC, HW], BF16, tag="x")
            h8 = hpool.tile([P, NCC, HW], FP8, tag="h8")
            k8 = kpool.tile([P, NCC, HW], FP8, tag="k8")
            q8 = qpool.tile([P, NCC, IHALF], FP8, tag="q8")
            vt8 = vpool.tile([P, NJC, C], FP8, tag="vt8")

            # ---- DMAs: x chunks on sync/gpsimd/scalar; weights+consts follow
            half = HW // 2
            for ci, eng in ((0, nc.sync), (1, nc.gpsimd), (2, nc.scalar),
                            (3, nc.sync)):
                eng.dma_start(out=x_sb[:, ci, :half], in_=x_d[ci * P:(ci + 1) * P, :half])
                eng.dma_start(out=x_sb[:, ci, half:], in_=x_d[ci * P:(ci + 1) * P, half:])
            for t, d in ((wq8, wq_d), (wk8, wk_d), (wv8, wv_d), (wp8, wp_d)):
                nc.sync.dma_start(out=t[:], in_=d[:].rearrange("(cc p) o -> p cc o", p=P))
            nc.gpsimd.dma_start(out=consts[:], in_=consts_d[:])
            nc.gpsimd.dma_start(out=bcast16[:], in_=bcast16_d[:])
            nc.gpsimd.dma_start(out=ones8[:], in_=ones8_d[:])
            nc.vector.memset(ones_bf[:], 1.0)
            nc.vector.memset(eps_sb[:], EPS)
            nc.vector.memset(ebias[:], EXP_BIAS)

            # ====== GroupNorm ======
            # stats: DVE bn_stats for chunks 0,1,3; ACT copy/square-accum for
            # chunk 2 (runs in parallel with DVE, ACT is idle pre-attention).
            # h8 = x*sc+sh -> fp8: chunks 2,0,1 on Pool, chunk 3 on DVE, so
            # both DoubleRow chunk-pairs (0,1) and (2,3) complete ~21.5us.
            with (
                tc.tile_pool(name="gn", bufs=2) as gpool,
                tc.tile_pool(name="gnp", bufs=2, space="PSUM") as gpsum_pool,
            ):
                gpsum = gpsum_pool.tile([P // GS, 2 * NCC], F32, tag="gstat")
                sc_all = gpool.tile([P, NCC], F32, tag="scall")
                sh_all = gpool.tile([P, NCC], F32, tag="shall")

                def finish_chunk(ci, t2):
                    nc.tensor.matmul(
                        gpsum[:, ci * 2:(ci + 1) * 2], lhsT=ind16, rhs=t2[:],
                        start=True, stop=True,
                    )
                    # group mean / rstd -> per-channel scale/shift
                    gmr = gpool.tile([P // GS, 2], F32, tag="gmr", name=f"gmr{ci}")
                    nc.vector.tensor_copy(out=gmr[:], in_=gpsum[:, ci * 2:(ci + 1) * 2])
                    mu = gmr[:, 0:1]
                    var = gmr[:, 1:2]
                    tmpv = gpool.tile([P // GS, 1], F32, tag="tmpv")
                    nc.vector.tensor_tensor(tmpv[:], mu, mu, ALU.mult)
                    nc.vector.tensor_tensor(var, var, tmpv[:], ALU.subtract)
                    nc.scalar.activation(
                        out=var, in_=var, func=ACTF.Sqrt, bias=eps_sb[:], scale=1.0,
                    )
                    nc.vector.reciprocal(out=var, in_=var)
                    bpsum = gpsum_pool.tile([P, 2], F32, tag="bc")
                    nc.tensor.matmul(
                        bpsum[:], lhsT=bcast16[:], rhs=gmr[:], start=True, stop=True,
                    )
                    sc = sc_all[:, ci:ci + 1]
                    sh = sh_all[:, ci:ci + 1]
                    nc.vector.tensor_tensor(sc, bpsum[:, 1:2], gamma[:, ci:ci + 1], ALU.mult)
                    nc.vector.tensor_tensor(sh, bpsum[:, 0:1], sc, ALU.mult)
                    nc.vector.tensor_tensor(sh, beta[:, ci:ci + 1], sh, ALU.subtract)
                    if ci == 2:
                        # Identity is present in every act table -> no reload
                        nc.scalar.activation(
                            out=h8[:, ci, :], in_=x_sb[:, ci, :],
                            func=ACTF.Identity, bias=sh, scale=sc,
                        )
                    elif ci == 3:
                        nc.vector.tensor_scalar(
                            out=h8[:, ci, :], in0=x_sb[:, ci, :],
                            scalar1=sc, scalar2=sh, op0=ALU.mult, op1=ALU.add,
                        )
                    else:
                        nc.gpsimd.tensor_scalar(
                            out=h8[:, ci, :], in0=x_sb[:, ci, :],
                            scalar1=sc, scalar2=sh, op0=ALU.mult, op1=ALU.add,
                        )

                # chunk 2 stats split: sum(x) on Pool, sum(x^2) on ACT (one
                # Square pass, so the GN sqrt chain is barely delayed); DVE
                # keeps bn_stats for chunks 0/1/3.  Scratch writes land in
                # h8[:,2,:] / k8[:,2,:512*2], later overwritten.
                s12 = gpool.tile([P, 2], F32, tag="s12")
                nc.gpsimd.tensor_scalar(
                    out=h8[:, 2, :], in0=x_sb[:, 2, :],
                    scalar1=1.0, scalar2=None, op0=ALU.mult, op1=ALU.add,
                    accum_out=s12[:, 0:1],
                )
                nc.scalar.activation(
                    out=k8[:, 2, :HW], in_=x_sb[:, 2, :],
                    func=ACTF.Square, accum_out=s12[:, 1:2],
                )
                t2c2 = gpool.tile([P, 2], F32, tag="t2c2")
                nc.vector.tensor_scalar_mul(t2c2[:], s12[:], 1.0 / HW)

                for ci in (0, 1, 3):
                    stats = gpool.tile([P, HW // 512, 6], F32, tag="stats")
                    for sg in range(HW // 512):
                        nc.vector.bn_stats(
                            out=stats[:, sg, :],
                            in_=x_sb[:, ci, sg * 512:(sg + 1) * 512],
                        )
                    mv = gpool.tile([P, 2], F32, tag="mv")
                    nc.vector.bn_aggr(out=mv[:], in_=stats[:])
                    t2 = gpool.tile([P, 2], F32, tag="t2")
                    nc.vector.tensor_copy(out=t2[:, 0:1], in_=mv[:, 0:1])
                    nc.vector.tensor_tensor(
                        t2[:, 1:2], mv[:, 0:1], mv[:, 0:1], ALU.mult
                    )
                    nc.vector.tensor_add(t2[:, 1:2], t2[:, 1:2], mv[:, 1:2])
                    finish_chunk(ci, t2)
                    if ci == 1:
                        finish_chunk(2, t2c2)
                # preload the exp activation table after the last Sqrt (the
                # input dep on sc_all pins it there despite list scheduling)
                expwarm = gpool.tile([P, 1], F32, tag="expwarm")
                nc.scalar.activation(
                    out=expwarm[:], in_=sc_all[:, 3:4], func=ACTF.Exp, scale=1.0,
                )

            # ====== convs + attention (fused pipeline, all fp8 DoubleRow) =====
            with (
                tc.tile_pool(name="et", bufs=4) as etpool,
                tc.tile_pool(name="o8b", bufs=2) as o8pool,
                tc.tile_pool(name="lb", bufs=2) as lbpool,
                tc.tile_pool(name="xpb", bufs=4) as xpbpool,
                tc.tile_pool(name="yt", bufs=4) as ytpool,
                tc.tile_pool(name="stp", bufs=3, space="PSUM") as stpool,
                tc.tile_pool(name="oap", bufs=1, space="PSUM") as oapool,
                tc.tile_pool(name="lp", bufs=1, space="PSUM") as lpool,
            ):
                def emit_q(ib):
                    isl = slice(ib * 512, (ib + 1) * 512)
                    for oc in range(NCC):
                        ps = stpool.tile([P, 512], F32, tag="st", name=f"q{ib}{oc}")
                        for p2 in range(NP2):
                            nc.tensor.matmul(
                                ps[:],
                                lhsT=wq8[:, 2 * p2:2 * p2 + 2, oc * P:(oc + 1) * P],
                                rhs=h8[:, 2 * p2:2 * p2 + 2, isl],
                                start=(p2 == 0), stop=(p2 == NP2 - 1),
                                perf_mode=DR,
                            )
                        qeng = nc.gpsimd if oc % 2 == 0 else nc.vector
                        qeng.tensor_scalar(
                            out=q8[:, oc, isl], in0=ps[:],
                            scalar1=bq16[:, oc:oc + 1], scalar2=None, op0=ALU.add,
                        )

                def emit_k(jt, ocs=(0, 1, 2, 3)):
                    tsl = slice(jt * 512, (jt + 1) * 512)
                    for oc in ocs:
                        ps = stpool.tile([P, 512], F32, tag="st", name=f"k{jt}{oc}")
                        for p2 in range(NP2):
                            nc.tensor.matmul(
                                ps[:],
                                lhsT=wk8[:, 2 * p2:2 * p2 + 2, oc * P:(oc + 1) * P],
                                rhs=h8[:, 2 * p2:2 * p2 + 2, tsl],
                                start=(p2 == 0), stop=(p2 == NP2 - 1),
                                perf_mode=DR,
                            )
                        # spread drains so no single engine paces the psum ring
                        eng = nc.vector if oc % 2 == 0 else nc.gpsimd
                        eng.tensor_scalar(
                            out=k8[:, oc, tsl], in0=ps[:],
                            scalar1=bk16[:, oc:oc + 1], scalar2=None, op0=ALU.add,
                        )

                def emit_v(jc):
                    ps = stpool.tile([P, 512], F32, tag="st", name=f"v{jc}")
                    for p2 in range(NP2):
                        nc.tensor.matmul(
                            ps[:],
                            lhsT=h8[:, 2 * p2:2 * p2 + 2, jc * P:(jc + 1) * P],
                            rhs=wv8[:, 2 * p2:2 * p2 + 2, :],
                            start=(p2 == 0), stop=(p2 == NP2 - 1),
                            perf_mode=DR,
                        )
                    eng = nc.gpsimd if jc % 2 == 0 else nc.vector
                    eng.tensor_copy(out=vt8[:, jc, :], in_=ps[:])

                def emit_proj(ib, o8t):
                    isl = slice(ib * 512, (ib + 1) * 512)
                    for oc in range(NCC):
                        xpb = xpbpool.tile([P, 512], F32, tag="xpb",
                                           name=f"xpb{ib}{oc}")
                        nc.gpsimd.tensor_scalar(
                            out=xpb[:], in0=x_sb[:, oc, isl],
                            scalar1=bppc[:, oc:oc + 1], scalar2=None, op0=ALU.add,
                        )
                        ps = stpool.tile([P, 512], F32, tag="st", name=f"p{ib}{oc}")
                        for p2 in range(NP2):
                            nc.tensor.matmul(
                                ps[:],
                                lhsT=wp8[:, 2 * p2:2 * p2 + 2, oc * P:(oc + 1) * P],
                                rhs=o8t[:, 2 * p2:2 * p2 + 2, :],
                                start=(p2 == 0), stop=(p2 == NP2 - 1),
                                perf_mode=DR,
                            )
                        eng = nc.vector if oc % 2 == 0 else nc.gpsimd
                        yt = ytpool.tile([P, 512], F32, tag="yt", name=f"yt{ib}{oc}")
                        eng.scalar_tensor_tensor(
                            out=yt[:], in0=ps[:], scalar=PROJ_SCALE, in1=xpb[:],
                            op0=ALU.mult, op1=ALU.add,
                        )
                        nc.sync.dma_start(out=y_d[oc * P:(oc + 1) * P, isl], in_=yt[:])

                emit_k(0)
                emit_q(0)
                emit_v(0)
                emit_q(1)

                # ---- single S/exp stream across all blocks (ACT never waits
                # on O-banks); AV/l trail by DEPTH slots; per-block epilogue
                # (lrb, o8) runs during the next block's exp window.
                DEPTH = 3
                slots = [(ib, jp) for ib in range(NBLK) for jp in range(NJP)]
                ets = {}
                opsums = {}
                lpsums = {}
                pending_proj = []

                def emit_s(ib, jp):
                    isl = slice(ib * 512, (ib + 1) * 512)
                    etp = etpool.tile([P, 2, 512], FP8, tag="et",
                                      name=f"et{ib}_{jp}")
                    for par in range(2):
                        jc = 2 * jp + par
                        ps = stpool.tile([P, 512], F32, tag="st",
                                         name=f"s{ib}_{jc}")
                        for p2 in range(NP2):
                            nc.tensor.matmul(
                                ps[:],
                                lhsT=k8[:, 2 * p2:2 * p2 + 2, jc * P:(jc + 1) * P],
                                rhs=q8[:, 2 * p2:2 * p2 + 2, isl],
                                start=(p2 == 0), stop=(p2 == NP2 - 1),
                                perf_mode=DR,
                            )
                        nc.scalar.activation(
                            out=etp[:, par, :], in_=ps[:],
                            func=ACTF.Exp, scale=SCALE_S, bias=ebias[:],
                        )
                    ets[(ib, jp)] = etp

                def emit_av(ib, jp):
                    if jp == 0:
                        opsums[ib] = [
                            oapool.tile([P, 512], F32, tag=f"o{cc}",
                                        name=f"ops{ib}{cc}")
                            for cc in range(NCC)
                        ]
                        lpsums[ib] = lpool.tile([1, 512], F32, tag="l",
                                                name=f"l{ib}")
                    etp = ets.pop((ib, jp))
                    for cc in range(NCC):
                        nc.tensor.matmul(
                            opsums[ib][cc][:],
                            lhsT=vt8[:, 2 * jp:2 * jp + 2, cc * P:(cc + 1) * P],
                            rhs=etp[:],
                            start=(jp == 0), stop=(jp == NJP - 1),
                            perf_mode=DR,
                        )
                    nc.tensor.matmul(
                        lpsums[ib][:], lhsT=ones8[:], rhs=etp[:],
                        start=(jp == 0), stop=(jp == NJP - 1),
                        perf_mode=DR,
                    )
                    if jp == NJP - 1:
                        finish_block(ib)

                def finish_block(ib):
                    # lrb = 64/sum(e^S) broadcast via ones-matmul, copied to
                    # SBUF quickly so the psum ring slot frees early
                    l_bf = lbpool.tile([1, 512], BF16, tag="lbf", name=f"lbf{ib}")
                    with nc.allow_low_precision(reason="1/l broadcast via bf16 matmul; 0.4% on a 6.5%-of-norm term"):
                        nc.vector.reciprocal(out=l_bf[:], in_=lpsums[ib][:])
                    lrbps = stpool.tile([P, 512], F32, tag="st", name=f"lrb{ib}")
                    nc.tensor.matmul(
                        lrbps[:], lhsT=ones_bf[:], rhs=l_bf[:], start=True, stop=True,
                    )
                    lrb = lbpool.tile([P, 512], BF16, tag="lrbsb", name=f"lrbsb{ib}")
                    nc.vector.tensor_copy(out=lrb[:], in_=lrbps[:])
                    o8t = o8pool.tile([P, NCC, 512], FP8, tag="o8", name=f"o8_{ib}")
                    for cc in range(NCC):
                        eng = nc.vector if cc % 2 == 0 else nc.gpsimd
                        eng.tensor_tensor(
                            o8t[:, cc, :], opsums[ib][cc][:], lrb[:], ALU.mult
                        )
                    pending_proj.append((ib, o8t))

                av_done = 0

                def drain_av(upto):
                    nonlocal av_done
                    while av_done < upto:
                        emit_av(*slots[av_done])
                        av_done += 1

                for s, (ib, jp) in enumerate(slots):
                    if ib == 0:
                        # just-in-time k/v conv work spread one half-k and one
                        # v chunk per slot (k jt0 + v0 emitted in the prologue)
                        jt, half = divmod(jp + 2, 2)
                        if jt <= NJT - 1:
                            emit_k(jt, (0, 1) if half == 0 else (2, 3))
                        if 2 * jp + 1 < NJC:
                            emit_v(2 * jp + 1)
                        if 2 * jp + 2 < NJC:
                            emit_v(2 * jp + 2)
                    if (ib, jp) == (1, 0):
                        emit_q(2)
                    if (ib, jp) == (1, 8):
                        emit_q(3)
                    emit_s(ib, jp)
                    # AV trails by DEPTH; the last block tapers to depth 1 so
                    # the epilogue (lrb -> o8 -> proj) starts early
                    depth = 1 if ib == NBLK - 1 else DEPTH
                    drain_av(max(0, s + 1 - depth))
                    if pending_proj and jp in (6, 7, 8, 9):
                        emit_proj(*pending_proj.pop(0))
                drain_av(len(slots))
                while pending_proj:
                    emit_proj(*pending_proj.pop(0))

    if split_waits:
        _split_excess_waits(nc)
    return nc


_NC = None


def _get_nc():
    global _NC
    if _NC is None:
        _NC = build_nc()
    return _NC


def _core0_feed(inputs):
    """Input map for core 0 (batch 0, first query half) — used by test harnesses."""
    maps = _build_in_maps(**inputs)
    return maps[0]


def _build_in_maps(x, gamma, beta, Wq, bq, Wk, bk, Wv, bv, Wp, bp):
    x = np.asarray(x, dtype=np.float32)
    B, c, H, W = x.shape
    assert (B, c, H, W) == (4, C, 64, 64)

    def pc(v):  # [C] -> [P, NCC]
        return np.ascontiguousarray(np.asarray(v, np.float32).reshape(NCC, P).T)

    ind16 = np.zeros((P, P // GS), np.float32)
    ind16[np.arange(P), np.arange(P) // GS] = 1.0 / GS
    bcast16 = np.zeros((P // GS, P), np.float32)
    bcast16[np.arange(P) // GS, np.arange(P)] = 1.0

    bp_eff = np.asarray(bp, np.float64) + np.asarray(Wp, np.float64) @ np.asarray(bv, np.float64)
    consts = np.concatenate(
        [pc(16.0 * np.asarray(bq, np.float32)),
         pc(16.0 * np.asarray(bk, np.float32)),
         pc(bp_eff.astype(np.float32)),
         pc(gamma), pc(beta), ind16], axis=1,
    ).astype(np.float32)

    def w8(w):
        return np.ascontiguousarray(16.0 * np.asarray(w, np.float32).T).astype(E4)

    shared = {
        "wq8": w8(Wq), "wk8": w8(Wk), "wv8": w8(Wv), "wp8": w8(Wp),
        "consts": np.ascontiguousarray(consts),
        "bcast16": bcast16,
        "ones8": np.full((P, 2, 1), ONES_VAL, E4),
    }

    xf = x.reshape(B, C, HW)
    in_maps = []
    for core in range(8):
        b, half = divmod(core, 2)
        xb = xf[b]
        if half == 0:
            x_bc = xb
        else:
            x_bc = np.concatenate([xb[:, IHALF:], xb[:, :IHALF]], axis=1)
        in_maps.append({"x_bf": np.ascontiguousarray(x_bc).astype(BF), **shared})
    return in_maps


def kernel(x, gamma, beta, Wq, bq, Wk, bk, Wv, bv, Wp, bp):
    nc = _get_nc()
    in_maps = _build_in_maps(x, gamma, beta, Wq, bq, Wk, bk, Wv, bv, Wp, bp)

    from concourse.bass_utils import run_bass_kernel_spmd

    res = run_bass_kernel_spmd(nc, in_maps, list(range(8)))

    B = 4
    out = np.empty((B, C, HW), np.float32)
    for core in range(8):
        b, half = divmod(core, 2)
        out[b, :, half * IHALF:(half + 1) * IHALF] = res.results[core]["yout"]
    return out.reshape(B, C, 64, 64)
